# revision 25
# baseline (speedup 1.0000x reference)
"""Multi-head causal attention (B=16, T=512, D=1024, H=16) on 8 TRN2 cores.

Sharding: data-parallel over batch (2 batches per core), weights replicated.
Per-core kernel (matmuls in float32r for full PE rate):
  x -> x^T (PE transpose) -> Q^T,K^T (feature-major) and V (token-major)
  S = Q_h^T.T @ K_h^T per head (causal: only lower-triangular blocks)
  P = exp(S/32 + mask) with row-sums from activation accum, P *= 1/sums
  P^T via PE transpose;  y^T = V_h.T @ P^T ;  out = y^T.T @ w_o + b_o
QKV projection streams weight k-tiles with 8 open PSUM accumulations
(ko-outer) so DMA overlaps compute; attention(b1) is interleaved with the
output projection of b0 to overlap ACT-bound softmax with PE-bound matmuls.
"""

import sys

sys.path.insert(0, "/opt/trn_rl_repo")

import numpy as np

B, T, D = 16, 512, 1024
H = 16
HD = D // H          # 64
NCORES = 8
BL = B // NCORES     # 2 local batches per core
PPART = 128

_CACHE = {}


def _build_program(reps=1, phases="xqaw"):
    import concourse.bass as bass
    import concourse.tile as tile
    from concourse import bacc, mybir
    from concourse.masks import make_identity

    DT = mybir.dt.float32
    R = mybir.dt.float32r
    ACTF = mybir.ActivationFunctionType
    ALU = mybir.AluOpType

    nc = bacc.Bacc("TRN2", target_bir_lowering=False, debug=False,
                   num_devices=NCORES)

    x_d = nc.dram_tensor("x", [BL, T, D], DT, kind="ExternalInput").ap()
    wqkv_d = nc.dram_tensor("w_qkv", [D, 3 * D], DT, kind="ExternalInput").ap()
    bqkv_d = nc.dram_tensor("b_qkv", [3 * D], DT, kind="ExternalInput").ap()
    wo_d = nc.dram_tensor("w_o", [D, D], DT, kind="ExternalInput").ap()
    bo_d = nc.dram_tensor("b_o", [D], DT, kind="ExternalInput").ap()
    out_d = nc.dram_tensor("out", [BL, T, D], DT, kind="ExternalOutput").ap()

    x_f = x_d.flatten_outer_dims()      # [1024, 1024] tokens x features
    out_fs = [out_d.flatten_outer_dims()]
    for r in range(1, reps):
        scr = nc.dram_tensor(f"scratch{r}", [BL, T, D], DT).ap()
        out_fs.append(scr.flatten_outer_dims())

    def f32r(ap):
        return ap.bitcast(R)

    with tile.TileContext(nc) as tc:
        with (
            tc.tile_pool(name="consts", bufs=1) as consts,
            tc.tile_pool(name="y", bufs=1) as y_pool,
            tc.tile_pool(name="xt", bufs=1) as xt_pool,
            tc.tile_pool(name="qkv", bufs=1) as qkv_pool,
            tc.tile_pool(name="w", bufs=9) as w_pool,
            tc.tile_pool(name="xn", bufs=2) as xn_pool,
            tc.tile_pool(name="pp", bufs=8) as p_pool,
            tc.tile_pool(name="pt", bufs=1) as pt_pool,
            tc.tile_pool(name="ss", bufs=6) as s_pool,
            tc.tile_pool(name="ob", bufs=3) as o_pool,
        ):
            # ---------------- constants ----------------
            ident_f = consts.tile([PPART, PPART], DT)
            make_identity(nc, ident_f)
            ident = consts.tile([PPART, PPART], R)
            nc.vector.tensor_copy(out=ident, in_=ident_f)

            maskneg = consts.tile([PPART, PPART], DT)
            nc.vector.memset(maskneg, 0.0)
            # iota = i - j ; keep 0.0 where j <= i, else -1e30 (causal)
            nc.gpsimd.affine_select(
                out=maskneg, in_=maskneg,
                compare_op=ALU.is_ge, fill=-1e30,
                base=0, pattern=[[-1, PPART]], channel_multiplier=1,
            )
            # bf16 copies so the mask can be added in PSUM by a PE matmul
            BF = mybir.dt.bfloat16
            mask_b = consts.tile([PPART, PPART], BF)
            nc.vector.tensor_copy(out=mask_b, in_=maskneg)
            ident_b = consts.tile([PPART, PPART], BF)
            nc.vector.tensor_copy(out=ident_b, in_=ident_f)

            ones_f = consts.tile([1, PPART], DT)
            nc.vector.memset(ones_f, 1.0)
            ones_row = consts.tile([1, PPART], R)
            nc.vector.tensor_copy(out=ones_row, in_=ones_f)

            # full qkv bias row + output bias row (rank-1 matmul adds)
            b_sb = consts.tile([1, 3 * D], R)
            bo_sb = consts.tile([1, D], R)
            ones_tok = consts.tile([1, T], R)

            def load_biases():
                nc.sync.dma_start(
                    out=b_sb,
                    in_=f32r(bqkv_d.rearrange("(a f) -> a f", a=1)))
                nc.sync.dma_start(
                    out=bo_sb, in_=f32r(bo_d.rearrange("(a f) -> a f", a=1)))
                ot_f = consts.tile([1, T], DT, name="ot_f")
                nc.vector.memset(ot_f, 1.0)
                nc.vector.tensor_copy(out=ones_tok, in_=ot_f)

            y_t = y_pool.tile([PPART, 8, BL * T], R)  # [128, 8, 1024]

            def load_x_and_transpose(b, trps):
                x_t = xt_pool.tile([PPART, 8, T], R)
                for to in range(4):
                    xn = xn_pool.tile([PPART, D], R)
                    nc.sync.dma_start(
                        out=xn,
                        in_=f32r(x_f[T * b + 128 * to:T * b + 128 * (to + 1), :]))
                    for fg in range(2):  # 4 transposes share one PSUM bank
                        pst = trps.tile([PPART, 4, PPART], R, tag="tr")
                        for fi in range(4):
                            fo = 4 * fg + fi
                            nc.tensor.transpose(
                                pst[:, fi, :], xn[:, 128 * fo:128 * (fo + 1)],
                                ident)
                        nc.vector.tensor_copy(
                            out=x_t[:, 4 * fg:4 * (fg + 1),
                                    128 * to:128 * (to + 1)],
                            in_=pst)
                return x_t

            def qkv_proj(b, x_t, qps):
                q_t = qkv_pool.tile([PPART, 8, T], R, tag="q")
                k_t = qkv_pool.tile([PPART, 8, T], R, tag="k")
                v_t = qkv_pool.tile([PPART, 4, D], R, tag="v")

                for sec in range(2):  # 0 -> Q^T, 1 -> K^T; ko-outer streaming
                    psums = [qps.tile([PPART, T], DT, tag="ps", name=f"qkvps{i}")
                              for i in range(8)]
                    for fo in range(8):
                        nc.tensor.matmul(
                            psums[fo],
                            lhsT=b_sb[:, 1024 * sec + 128 * fo:
                                      1024 * sec + 128 * (fo + 1)],
                            rhs=ones_tok,
                            start=True, stop=False)
                    for ko in range(8):
                        w_sb = w_pool.tile([PPART, 1024], R, tag="w")
                        nc.sync.dma_start(
                            out=w_sb,
                            in_=f32r(wqkv_d[128 * ko:128 * (ko + 1),
                                            1024 * sec:1024 * (sec + 1)]))
                        for fo in range(8):
                            nc.tensor.matmul(
                                psums[fo],
                                lhsT=w_sb[:, 128 * fo:128 * (fo + 1)],
                                rhs=x_t[:, ko, :],
                                start=False, stop=(ko == 7))
                    dst = q_t if sec == 0 else k_t
                    for fo in range(8):
                        nc.scalar.activation(
                            out=dst[:, fo, :], in_=psums[fo], func=ACTF.Copy)

                psums = [qps.tile([PPART, T], DT, tag="ps", name=f"qkvps{i}")
                              for i in range(8)]
                for ko in range(8):
                    w_sb = w_pool.tile([PPART, 1024], R, tag="w")
                    nc.sync.dma_start(
                        out=w_sb,
                        in_=f32r(wqkv_d[128 * ko:128 * (ko + 1), 2048:3072]))
                    for to in range(4):
                        for nh in range(2):
                            nc.tensor.matmul(
                                psums[2 * to + nh],
                                lhsT=x_t[:, ko, 128 * to:128 * (to + 1)],
                                rhs=w_sb[:, 512 * nh:512 * (nh + 1)],
                                start=(ko == 0), stop=False)
                for to in range(4):
                    for nh in range(2):
                        nc.tensor.matmul(
                            psums[2 * to + nh], lhsT=ones_row,
                            rhs=b_sb[:, 2 * D + 512 * nh:2 * D + 512 * (nh + 1)],
                            start=False, stop=True)
                        nc.vector.tensor_copy(
                            out=v_t[:, to, 512 * nh:512 * (nh + 1)],
                            in_=psums[2 * to + nh])
                return q_t, k_t, v_t

            def attn_pair(b, j, q_t, k_t, v_t, sps, trps, yps):
                for base in (0, 64):  # head 2j (base 0), 2j+1 (base 64)
                    h = 2 * j + (base // 64)
                    psy = yps.tile([64, T], DT, tag="y")
                    sums = s_pool.tile([PPART, 4], DT, tag="sums")
                    rsum = s_pool.tile([PPART, 4], DT, tag="rsum")
                    pchunks = []
                    for qt in range(4):
                        np_ = 128 * (qt + 1)
                        ps = sps.tile([PPART, T], DT, tag="s")
                        nc.tensor.matmul(
                            ps[:, :np_],
                            lhsT=q_t[base:base + 64, j,
                                     128 * qt:128 * (qt + 1)],
                            rhs=k_t[base:base + 64, j, :np_],
                            start=True, stop=False)
                        nc.tensor.matmul(
                            ps[:, 128 * qt:np_],
                            lhsT=ident_b, rhs=mask_b,
                            start=False, stop=True)
                        pch = p_pool.tile([PPART, T], R, tag="P")
                        nc.scalar.activation(
                            out=pch[:, :np_], in_=ps[:, :np_],
                            func=ACTF.Exp, scale=1.0 / 32.0,
                            accum_out=sums[:, qt:qt + 1])
                        pchunks.append(pch)
                    nc.vector.reciprocal(rsum, sums)
                    ptile = pt_pool.tile([PPART, 4, T], R,
                                         tag="PTa" if base == 0 else "PTb")
                    for qt in range(4):
                        np_ = 128 * (qt + 1)
                        nc.scalar.activation(
                            out=pchunks[qt][:, :np_],
                            in_=pchunks[qt][:, :np_],
                            func=ACTF.Copy, scale=rsum[:, qt:qt + 1])
                        pst = trps.tile([PPART, 4, PPART], R, tag="tr")
                        for kt in range(qt + 1):
                            nc.tensor.transpose(
                                pst[:, kt, :],
                                pchunks[qt][:, 128 * kt:128 * (kt + 1)],
                                ident)
                        nc.vector.tensor_copy(
                            out=ptile[:, :qt + 1, 128 * qt:128 * (qt + 1)],
                            in_=pst[:, :qt + 1, :])
                    for kt in range(4):
                        nc.tensor.matmul(
                            psy[:, 128 * kt:],
                            lhsT=v_t[:, kt, 64 * h:64 * (h + 1)],
                            rhs=ptile[:, kt, 128 * kt:],
                            start=(kt == 0), stop=(kt == 3))
                    nc.vector.tensor_copy(
                        out=y_t[base:base + 64, j, T * b:T * (b + 1)],
                        in_=psy)

            wo_tiles = {}

            def load_wo():
                for ko in range(8):
                    w_sb = w_pool.tile([PPART, 1024], R, tag="w")
                    nc.sync.dma_start(
                        out=w_sb, in_=f32r(wo_d[128 * ko:128 * (ko + 1), :]))
                    wo_tiles[ko] = w_sb

            def wo_chunk(tg, nh, sps, out_f):
                ps = sps.tile([PPART, T], DT, tag="s")
                for ko in range(8):
                    nc.tensor.matmul(
                        ps,
                        lhsT=y_t[:, ko, 128 * tg:128 * (tg + 1)],
                        rhs=wo_tiles[ko][:, 512 * nh:512 * (nh + 1)],
                        start=(ko == 0), stop=False)
                nc.tensor.matmul(
                    ps, lhsT=ones_row,
                    rhs=bo_sb[:, 512 * nh:512 * (nh + 1)],
                    start=False, stop=True)
                ob = o_pool.tile([PPART, T], DT)
                nc.scalar.activation(out=ob, in_=ps, func=ACTF.Copy)
                nc.sync.dma_start(
                    out=out_f[128 * tg:128 * (tg + 1),
                              512 * nh:512 * (nh + 1)], in_=ob)

            # ---------------- schedule ----------------
            for rep in range(reps):
              out_f = out_fs[rep]
              sfx = str(rep)
              with tc.tile_pool(name="ps0" + sfx, bufs=2, space="PSUM") as trps:
                x_t = load_x_and_transpose(0, trps)
                if rep == 0:
                    load_biases()
              if "q" not in phases:
                continue
              with tc.tile_pool(name="qps0" + sfx, bufs=8, space="PSUM") as qps:
                q_t, k_t, v_t = qkv_proj(0, x_t, qps)
              if "a" in phases:
                with (
                  tc.tile_pool(name="aps0" + sfx, bufs=3, space="PSUM") as sps,
                  tc.tile_pool(name="atr0" + sfx, bufs=3, space="PSUM") as trps,
                  tc.tile_pool(name="ay0" + sfx, bufs=2, space="PSUM") as yps,
                ):
                  for j in range(H // 2):
                      attn_pair(0, j, q_t, k_t, v_t, sps, trps, yps)
                  x_t = load_x_and_transpose(1, trps)
              else:
                with tc.tile_pool(name="xx0" + sfx, bufs=2, space="PSUM") as trps:
                  x_t = load_x_and_transpose(1, trps)
              with tc.tile_pool(name="qps1" + sfx, bufs=8, space="PSUM") as qps:
                q_t, k_t, v_t = qkv_proj(1, x_t, qps)
              if "a" not in phases:
                continue
              load_wo()
              with (
                tc.tile_pool(name="aps1" + sfx, bufs=3, space="PSUM") as sps,
                tc.tile_pool(name="atr1" + sfx, bufs=3, space="PSUM") as trps,
                tc.tile_pool(name="ay1" + sfx, bufs=2, space="PSUM") as yps,
              ):
                # attention(b1) interleaved with output projection of b0
                for j in range(H // 2):
                    attn_pair(1, j, q_t, k_t, v_t, sps, trps, yps)
                    if "w" in phases:
                        wo_chunk(j // 2, j % 2, sps, out_f)
                if "w" in phases:
                    for tg in range(4, 8):
                        for nh in range(2):
                            wo_chunk(tg, nh, sps, out_f)

    nc.compile()
    return nc


def _get_program(reps=1, phases="xqaw"):
    key = f"nc{reps}{phases}"
    if key not in _CACHE:
        _CACHE[key] = _build_program(reps, phases)
    return _CACHE[key]


def kernel(x, w_qkv, b_qkv, w_o, b_o):
    from concourse.bass_utils import run_bass_kernel_spmd

    nc = _get_program()
    x = np.ascontiguousarray(x, dtype=np.float32)
    in_maps = []
    for c in range(NCORES):
        in_maps.append({
            "x": x[BL * c:BL * (c + 1)],
            "w_qkv": np.asarray(w_qkv, dtype=np.float32),
            "b_qkv": np.asarray(b_qkv, dtype=np.float32),
            "w_o": np.asarray(w_o, dtype=np.float32),
            "b_o": np.asarray(b_o, dtype=np.float32),
        })
    res = run_bass_kernel_spmd(nc, in_maps, list(range(NCORES)))
    return np.concatenate([res.results[c]["out"] for c in range(NCORES)], axis=0)


# revision 30
# speedup vs baseline: 14.0246x; 14.0246x over previous
"""Multi-head causal attention (B=16, T=512, D=1024, H=16) on 8 TRN2 cores.

Sharding: data-parallel over batch (2 batches per core), weights replicated.
Per-core kernel (matmuls in float32r for full PE rate):
  x -> x^T (PE transpose) -> Q^T,K^T (feature-major) and V (token-major)
  S = Q_h^T.T @ K_h^T per head (causal: only lower-triangular blocks)
  P = exp(S/32 + mask) with row-sums from activation accum, P *= 1/sums
  P^T via PE transpose;  y^T = V_h.T @ P^T ;  out = y^T.T @ w_o + b_o
QKV projection streams weight k-tiles with 8 open PSUM accumulations
(ko-outer) so DMA overlaps compute; attention(b1) is interleaved with the
output projection of b0 to overlap ACT-bound softmax with PE-bound matmuls.
"""

import sys

sys.path.insert(0, "/opt/trn_rl_repo")

import numpy as np

B, T, D = 16, 512, 1024
H = 16
HD = D // H          # 64
NCORES = 8
BL = B // NCORES     # 2 local batches per core
PPART = 128

_CACHE = {}


def _build_program(reps=1, phases="xqaw"):
    import concourse.bass as bass
    import concourse.tile as tile
    from concourse import bacc, mybir
    from concourse.masks import make_identity

    DT = mybir.dt.float32
    R = mybir.dt.float32r
    ACTF = mybir.ActivationFunctionType
    ALU = mybir.AluOpType

    nc = bacc.Bacc("TRN2", target_bir_lowering=False, debug=False,
                   num_devices=NCORES)

    x_d = nc.dram_tensor("x", [BL, T, D], DT, kind="ExternalInput").ap()
    wqkv_d = nc.dram_tensor("w_qkv", [D, 3 * D], DT, kind="ExternalInput").ap()
    bqkv_d = nc.dram_tensor("b_qkv", [3 * D], DT, kind="ExternalInput").ap()
    wo_d = nc.dram_tensor("w_o", [D, D], DT, kind="ExternalInput").ap()
    bo_d = nc.dram_tensor("b_o", [D], DT, kind="ExternalInput").ap()
    out_d = nc.dram_tensor("out", [BL, T, D], DT, kind="ExternalOutput").ap()

    x_f = x_d.flatten_outer_dims()      # [1024, 1024] tokens x features
    out_fs = [out_d.flatten_outer_dims()]
    for r in range(1, reps):
        scr = nc.dram_tensor(f"scratch{r}", [BL, T, D], DT).ap()
        out_fs.append(scr.flatten_outer_dims())

    def f32r(ap):
        return ap.bitcast(R)

    with tile.TileContext(nc) as tc:
        with (
            tc.tile_pool(name="consts", bufs=1) as consts,
            tc.tile_pool(name="y", bufs=1) as y_pool,
            tc.tile_pool(name="xt", bufs=1) as xt_pool,
            tc.tile_pool(name="qkv", bufs=1) as qkv_pool,
            tc.tile_pool(name="w", bufs=9) as w_pool,
            tc.tile_pool(name="xn", bufs=3) as xn_pool,
            tc.tile_pool(name="pp", bufs=8) as p_pool,
            tc.tile_pool(name="pt", bufs=1) as pt_pool,
            tc.tile_pool(name="ss", bufs=6) as s_pool,
            tc.tile_pool(name="ob", bufs=3) as o_pool,
        ):
            # ---------------- constants ----------------
            ident_f = consts.tile([PPART, PPART], DT)
            make_identity(nc, ident_f)
            ident = consts.tile([PPART, PPART], R)
            nc.vector.tensor_copy(out=ident, in_=ident_f)

            maskneg = consts.tile([PPART, PPART], DT)
            nc.vector.memset(maskneg, 0.0)
            # iota = i - j ; keep 0.0 where j <= i, else -1e30 (causal)
            nc.gpsimd.affine_select(
                out=maskneg, in_=maskneg,
                compare_op=ALU.is_ge, fill=-1e30,
                base=0, pattern=[[-1, PPART]], channel_multiplier=1,
            )
            # bf16 copies so the mask can be added in PSUM by a PE matmul
            BF = mybir.dt.bfloat16
            mask_b = consts.tile([PPART, PPART], BF)
            nc.vector.tensor_copy(out=mask_b, in_=maskneg)
            ident_b = consts.tile([PPART, PPART], BF)
            nc.vector.tensor_copy(out=ident_b, in_=ident_f)

            ones_f = consts.tile([1, PPART], DT)
            nc.vector.memset(ones_f, 1.0)
            ones_row = consts.tile([1, PPART], R)
            nc.vector.tensor_copy(out=ones_row, in_=ones_f)

            # full qkv bias row + output bias row (rank-1 matmul adds)
            b_sb = consts.tile([1, 3 * D], R)
            bo_sb = consts.tile([1, D], R)
            ones_tok = consts.tile([1, T], R)

            def load_biases():
                nc.sync.dma_start(
                    out=b_sb,
                    in_=f32r(bqkv_d.rearrange("(a f) -> a f", a=1)))
                nc.sync.dma_start(
                    out=bo_sb, in_=f32r(bo_d.rearrange("(a f) -> a f", a=1)))
                ot_f = consts.tile([1, T], DT, name="ot_f")
                nc.vector.memset(ot_f, 1.0)
                nc.vector.tensor_copy(out=ones_tok, in_=ot_f)

            y_t = y_pool.tile([PPART, 8, BL * T], R)  # [128, 8, 1024]

            def load_x_and_transpose(b, trps):
                x_t = xt_pool.tile([PPART, 8, T], R)
                for to in range(4):
                    xn = xn_pool.tile([PPART, D], R)
                    nc.sync.dma_start(
                        out=xn,
                        in_=f32r(x_f[T * b + 128 * to:T * b + 128 * (to + 1), :]))
                    for fg in range(2):  # 4 transposes share one PSUM bank
                        pst = trps.tile([PPART, 4, PPART], R, tag="tr")
                        for fi in range(4):
                            fo = 4 * fg + fi
                            nc.tensor.transpose(
                                pst[:, fi, :], xn[:, 128 * fo:128 * (fo + 1)],
                                ident)
                        nc.vector.tensor_copy(
                            out=x_t[:, 4 * fg:4 * (fg + 1),
                                    128 * to:128 * (to + 1)],
                            in_=pst)
                return x_t

            def preload_sec(sec):
                tiles = []
                for ko in range(8):
                    w_sb = w_pool.tile([PPART, 1024], R, tag="w",
                                       name=f"wpre{sec}_{ko}")
                    nc.sync.dma_start(
                        out=w_sb,
                        in_=f32r(wqkv_d[128 * ko:128 * (ko + 1),
                                        1024 * sec:1024 * (sec + 1)]))
                    tiles.append(w_sb)
                return tiles

            def qkv_proj(b, x_t, qps, pre=None):
                q_t = qkv_pool.tile([PPART, 8, T], R, tag="q")
                k_t = qkv_pool.tile([PPART, 8, T], R, tag="k")
                v_t = qkv_pool.tile([PPART, 4, D], R, tag="v")

                for sec in range(2):  # 0 -> Q^T, 1 -> K^T; ko-outer streaming
                    psums = [qps.tile([PPART, T], DT, tag="ps", name=f"qkvps{i}")
                              for i in range(8)]
                    for fo in range(8):
                        nc.tensor.matmul(
                            psums[fo],
                            lhsT=b_sb[:, 1024 * sec + 128 * fo:
                                      1024 * sec + 128 * (fo + 1)],
                            rhs=ones_tok,
                            start=True, stop=False)
                    for ko in range(8):
                        if sec == 0 and pre is not None:
                            w_sb = pre[ko]
                        else:
                            w_sb = w_pool.tile([PPART, 1024], R, tag="w")
                            nc.sync.dma_start(
                                out=w_sb,
                                in_=f32r(wqkv_d[128 * ko:128 * (ko + 1),
                                                1024 * sec:1024 * (sec + 1)]))
                        for fo in range(8):
                            nc.tensor.matmul(
                                psums[fo],
                                lhsT=w_sb[:, 128 * fo:128 * (fo + 1)],
                                rhs=x_t[:, ko, :],
                                start=False, stop=(ko == 7))
                    dst = q_t if sec == 0 else k_t
                    for fo in range(8):
                        nc.scalar.activation(
                            out=dst[:, fo, :], in_=psums[fo], func=ACTF.Copy)

                psums = [qps.tile([PPART, T], DT, tag="ps", name=f"qkvps{i}")
                              for i in range(8)]
                for ko in range(8):
                    w_sb = w_pool.tile([PPART, 1024], R, tag="w")
                    nc.sync.dma_start(
                        out=w_sb,
                        in_=f32r(wqkv_d[128 * ko:128 * (ko + 1), 2048:3072]))
                    for to in range(4):
                        for nh in range(2):
                            nc.tensor.matmul(
                                psums[2 * to + nh],
                                lhsT=x_t[:, ko, 128 * to:128 * (to + 1)],
                                rhs=w_sb[:, 512 * nh:512 * (nh + 1)],
                                start=(ko == 0), stop=False)
                for to in range(4):
                    for nh in range(2):
                        nc.tensor.matmul(
                            psums[2 * to + nh], lhsT=ones_row,
                            rhs=b_sb[:, 2 * D + 512 * nh:2 * D + 512 * (nh + 1)],
                            start=False, stop=True)
                        nc.scalar.activation(
                            out=v_t[:, to, 512 * nh:512 * (nh + 1)],
                            in_=psums[2 * to + nh], func=ACTF.Copy)
                return q_t, k_t, v_t

            def attn_pair(b, j, q_t, k_t, v_t, sps, trps, yps):
                for base in (0, 64):  # head 2j (base 0), 2j+1 (base 64)
                    h = 2 * j + (base // 64)
                    psy = yps.tile([64, T], DT, tag="y")
                    sums = s_pool.tile([PPART, 4], DT, tag="sums")
                    rsum = s_pool.tile([PPART, 4], DT, tag="rsum")
                    pchunks = []
                    for qt in range(4):
                        np_ = 128 * (qt + 1)
                        ps = sps.tile([PPART, T], DT, tag="s")
                        nc.tensor.matmul(
                            ps[:, :np_],
                            lhsT=q_t[base:base + 64, j,
                                     128 * qt:128 * (qt + 1)],
                            rhs=k_t[base:base + 64, j, :np_],
                            start=True, stop=False)
                        nc.tensor.matmul(
                            ps[:, 128 * qt:np_],
                            lhsT=ident_b, rhs=mask_b,
                            start=False, stop=True)
                        pch = p_pool.tile([PPART, T], R, tag="P")
                        nc.scalar.activation(
                            out=pch[:, :np_], in_=ps[:, :np_],
                            func=ACTF.Exp, scale=1.0 / 32.0,
                            accum_out=sums[:, qt:qt + 1])
                        pchunks.append(pch)
                    nc.vector.reciprocal(rsum, sums)
                    ptile = pt_pool.tile([PPART, 4, T], R,
                                         tag="PTa" if base == 0 else "PTb")
                    for qt in range(4):
                        np_ = 128 * (qt + 1)
                        nc.vector.tensor_scalar_mul(
                            out=pchunks[qt][:, :np_],
                            in0=pchunks[qt][:, :np_],
                            scalar1=rsum[:, qt:qt + 1])
                        pst = trps.tile([PPART, 4, PPART], R, tag="tr")
                        for kt in range(qt + 1):
                            nc.tensor.transpose(
                                pst[:, kt, :],
                                pchunks[qt][:, 128 * kt:128 * (kt + 1)],
                                ident)
                        nc.vector.tensor_copy(
                            out=ptile[:, :qt + 1, 128 * qt:128 * (qt + 1)],
                            in_=pst[:, :qt + 1, :])
                    for kt in range(4):
                        nc.tensor.matmul(
                            psy[:, 128 * kt:],
                            lhsT=v_t[:, kt, 64 * h:64 * (h + 1)],
                            rhs=ptile[:, kt, 128 * kt:],
                            start=(kt == 0), stop=(kt == 3))
                    nc.scalar.activation(
                        out=y_t[base:base + 64, j, T * b:T * (b + 1)],
                        in_=psy, func=ACTF.Copy)

            wo_tiles = {}

            def load_wo():
                for ko in range(8):
                    w_sb = w_pool.tile([PPART, 1024], R, tag="w")
                    nc.sync.dma_start(
                        out=w_sb, in_=f32r(wo_d[128 * ko:128 * (ko + 1), :]))
                    wo_tiles[ko] = w_sb

            def wo_chunk(tg, nh, sps, out_f):
                ps = sps.tile([PPART, T], DT, tag="s")
                for ko in range(8):
                    nc.tensor.matmul(
                        ps,
                        lhsT=y_t[:, ko, 128 * tg:128 * (tg + 1)],
                        rhs=wo_tiles[ko][:, 512 * nh:512 * (nh + 1)],
                        start=(ko == 0), stop=False)
                nc.tensor.matmul(
                    ps, lhsT=ones_row,
                    rhs=bo_sb[:, 512 * nh:512 * (nh + 1)],
                    start=False, stop=True)
                ob = o_pool.tile([PPART, T], DT)
                nc.scalar.activation(out=ob, in_=ps, func=ACTF.Copy)
                nc.sync.dma_start(
                    out=out_f[128 * tg:128 * (tg + 1),
                              512 * nh:512 * (nh + 1)], in_=ob)

            # ---------------- schedule ----------------
            for rep in range(reps):
              out_f = out_fs[rep]
              sfx = str(rep)
              with tc.tile_pool(name="ps0" + sfx, bufs=2, space="PSUM") as trps:
                x_t = load_x_and_transpose(0, trps)
                if rep == 0:
                    load_biases()
              if "q" not in phases:
                continue
              with tc.tile_pool(name="qps0" + sfx, bufs=8, space="PSUM") as qps:
                q_t, k_t, v_t = qkv_proj(0, x_t, qps)
              if "a" in phases:
                with (
                  tc.tile_pool(name="aps0" + sfx, bufs=3, space="PSUM") as sps,
                  tc.tile_pool(name="atr0" + sfx, bufs=3, space="PSUM") as trps,
                  tc.tile_pool(name="ay0" + sfx, bufs=2, space="PSUM") as yps,
                ):
                  x_t2 = load_x_and_transpose(1, trps)
                  pre = preload_sec(0)
                  for j in range(H // 2):
                      attn_pair(0, j, q_t, k_t, v_t, sps, trps, yps)
                  x_t = x_t2
              else:
                pre = None
                with tc.tile_pool(name="xx0" + sfx, bufs=2, space="PSUM") as trps:
                  x_t = load_x_and_transpose(1, trps)
              with tc.tile_pool(name="qps1" + sfx, bufs=8, space="PSUM") as qps:
                q_t, k_t, v_t = qkv_proj(1, x_t, qps, pre=pre)
              if "a" not in phases:
                continue
              load_wo()
              with (
                tc.tile_pool(name="aps1" + sfx, bufs=3, space="PSUM") as sps,
                tc.tile_pool(name="atr1" + sfx, bufs=3, space="PSUM") as trps,
                tc.tile_pool(name="ay1" + sfx, bufs=2, space="PSUM") as yps,
              ):
                # attention(b1) interleaved with output projection of b0
                for j in range(H // 2):
                    attn_pair(1, j, q_t, k_t, v_t, sps, trps, yps)
                    if "w" in phases:
                        wo_chunk(j // 2, j % 2, sps, out_f)
                if "w" in phases:
                    for tg in range(4, 8):
                        for nh in range(2):
                            wo_chunk(tg, nh, sps, out_f)

    nc.compile()
    return nc


def _get_program(reps=1, phases="xqaw"):
    key = f"nc{reps}{phases}"
    if key not in _CACHE:
        _CACHE[key] = _build_program(reps, phases)
    return _CACHE[key]


def kernel(x, w_qkv, b_qkv, w_o, b_o):
    from concourse.bass_utils import run_bass_kernel_spmd

    nc = _get_program()
    x = np.ascontiguousarray(x, dtype=np.float32)
    in_maps = []
    for c in range(NCORES):
        in_maps.append({
            "x": x[BL * c:BL * (c + 1)],
            "w_qkv": np.asarray(w_qkv, dtype=np.float32),
            "b_qkv": np.asarray(b_qkv, dtype=np.float32),
            "w_o": np.asarray(w_o, dtype=np.float32),
            "b_o": np.asarray(b_o, dtype=np.float32),
        })
    res = run_bass_kernel_spmd(nc, in_maps, list(range(NCORES)))
    return np.concatenate([res.results[c]["out"] for c in range(NCORES)], axis=0)
